# revision 7
# baseline (speedup 1.0000x reference)
"""Trainium2 Bass kernel for: 1x1-conv GEMM + GroupNorm + HardTanh.

Reference computation (per sample b):
    y = weight @ x[b]                        # [512, 256] @ [256, 56*56]
    groupnorm over 32 groups of 16 channels  # stats over (16, 56*56)
    y = y * gamma + beta                     # per-channel affine
    out = clip(y, -2, 2)

Sharding: data-parallel over batch, 4 samples per core x 8 cores.
weight/gamma/beta replicated. No cross-core communication needed.

HBM-bandwidth-bound by design: x and the output travel as fp16
(halving DMA traffic vs fp32; fp16's 10 mantissa bits keep the
end-to-end error ~1e-3 of scale) and the matmul runs in fp16 at the
full PE rate. Every 128-channel chunk runs its B half (cols 2048:3136)
first, so the DVE's immediate square+reduce work follows the short B
copy while the long A copy streams behind it. Engine assignment:
  PE   : 512-col matmuls into 4-bank B/A PSUM tiles (one rotating slot
         pair - chunk k+1 fills while k drains) + one tiny per-sample
         group-aggregation matmul.
  ACT  : sole PSUM reader - Copy PSUM fp32 -> SBUF fp16, accum_out
         yielding per-channel sum(y) as a free side effect.
  Pool : squares the A-half head columns (TensorTensor mult fp16).
  DVE  : squares the B half + A tail (TT 2x mode); in 4x mode the
         region sum(y^2) reduces, the affine, and the clamp.
The whole schedule is software-pipelined with uniform lags so no
in-order engine queue ever parks on a long dependency:
  - chunk g's A-half reduce runs at chunk g+1 (Pool gets a full chunk
    period to finish its squares);
  - sample b's group-aggregation matmul + scale/bias chain are emitted
    INSIDE chunk (b+1,0), writing the group stats into that chunk's
    own fresh psB tail columns (then the PE never idles on them);
  - transform (affine+clamp+store) of chunk g runs at chunk g+5.
Group stats are mathematically per-chunk (each 128-channel chunk holds
8 whole groups), aggregated per-sample only to amortize the chain.
The trailing sample squares everything on DVE and splits its clamps
DVE/Pool to compress the drain.
"""

import sys

sys.path.insert(0, "/opt/trn_rl_repo")

import numpy as np

import concourse.bacc as bacc
import concourse.mybir as mybir
import concourse.tile as tile
from concourse.bass_utils import run_bass_kernel_spmd

# Problem shape (hardcoded per contest contract)
B, CIN, COUT, H, W = 32, 256, 512, 56, 56
HW = H * W  # 3136
G = 32  # num groups
GSIZE = COUT // G  # 16 channels per group
EPS = 1e-5
HT_MIN, HT_MAX = -2.0, 2.0

N_CORES = 8
BPC = B // N_CORES  # samples per core = 4
KC = CIN // 128  # contraction chunks = 2
OC = COUT // 128  # output-channel chunks = 4
NCHUNK = BPC * OC  # 16

HWA = 2048  # A half: hw cols [0, 2048)
HWB = HW - HWA  # B half: [2048, 3136) = 1088 cols
A_TILES = [(t * 512, 512) for t in range(4)]
B_TILES = [(0, 512), (512, 512), (1024, 64)]
GPS_OFF = 1536  # group-stats scratch cols inside a B psum tile

TPOOL = 1740  # gpsimd square-head width (0 on the final chunk)
TLAG = 5  # transform(g) runs at chunk g+TLAG
DRAIN_POOL = 1024  # drain-phase clamp split

XQ = 4  # x loaded in 4 column-range DMAs so matmuls start early

_NC_CACHE = None


def _build_program():
    f32 = mybir.dt.float32
    f16 = mybir.dt.float16

    nc = bacc.Bacc("TRN2", target_bir_lowering=False, debug=False)

    x_d = nc.dram_tensor("x", [BPC, CIN, HW], f16, kind="ExternalInput")
    wt_d = nc.dram_tensor("wt", [CIN, COUT], f16, kind="ExternalInput")
    gamma_d = nc.dram_tensor("gamma", [COUT], f32, kind="ExternalInput")
    beta_d = nc.dram_tensor("beta", [COUT], f32, kind="ExternalInput")
    agg_d = nc.dram_tensor("agg", [128, 128], f32, kind="ExternalInput")
    out_d = nc.dram_tensor("out", [BPC, COUT, HW], f16, kind="ExternalOutput")

    with tile.TileContext(nc) as tc:
        with (
            tc.tile_pool(name="singles", bufs=1) as singles,
            tc.tile_pool(name="xp", bufs=2) as xp,
            tc.tile_pool(name="yp", bufs=8) as yp,
            tc.tile_pool(name="up", bufs=3) as up,
            tc.tile_pool(name="fp", bufs=4) as fp,
            tc.tile_pool(name="tp", bufs=3) as tp,
            tc.tile_pool(name="small", bufs=2) as small,
            tc.tile_pool(name="psy", bufs=2, space="PSUM") as psy,
        ):
            # --- one-time setup -------------------------------------------
            # sample 0's B-half x sliver first (the first matmuls read it),
            # then weights, then the rest of x; scalars ride SWDGE
            x0_sb = xp.tile([128, KC, HW], f16, tag="x")

            def load_x_part(x_tile, b, lo, hi):
                nc.sync.dma_start(
                    out=x_tile[:, :, lo:hi],
                    in_=x_d.ap()[b, :, lo:hi].rearrange(
                        "(c p) f -> p c f", p=128
                    ),
                )

            load_x_part(x0_sb, 0, HWA, HW)
            wt_sb = singles.tile([128, KC, COUT], f16)
            nc.sync.dma_start(
                out=wt_sb, in_=wt_d.ap().rearrange("(c p) m -> p c m", p=128)
            )
            load_x_part(x0_sb, 0, 0, 1024)
            load_x_part(x0_sb, 0, 1024, HWA)
            gamma_sb = singles.tile([128, OC], f32)
            nc.gpsimd.dma_start(
                out=gamma_sb, in_=gamma_d.ap().rearrange("(c p) -> p c", p=128)
            )
            beta_sb = singles.tile([128, OC], f32)
            nc.gpsimd.dma_start(
                out=beta_sb, in_=beta_d.ap().rearrange("(c p) -> p c", p=128)
            )
            eps_sb = singles.tile([128, 1], f32)
            nc.vector.memset(eps_sb, EPS)
            agg_sb = singles.tile([128, 128], f32)
            nc.gpsimd.dma_start(out=agg_sb, in_=agg_d.ap())

            x_tiles = [x0_sb]

            # pipeline state, keyed by global chunk index g = 4*b + oc
            y_tiles = {}
            y2_tiles = {}
            tp_w = {}  # g -> gpsimd head width used
            sums_t = {}  # b -> accumulator tile
            sb_t = {}  # b -> (s4, bv4)

            def mm_half(ps, tiles, base, x_sb, osl):
                for lo, wdt in tiles:
                    for c in range(KC):
                        nc.tensor.matmul(
                            ps[:, lo : lo + wdt],
                            wt_sb[:, c, osl],
                            x_sb[:, c, base + lo : base + lo + wdt],
                            start=(c == 0),
                            stop=(c == KC - 1),
                        )

            def emit_chunk(g):
                """matmuls + PSUM evacuation + squares + B-half reduce."""
                b, oc = divmod(g, OC)
                x_sb = x_tiles[b]
                osl = slice(oc * 128, (oc + 1) * 128)
                sums = sums_t[b]

                psB = psy.tile([128, 2048], f32, tag="ps")
                psA = psy.tile([128, 2048], f32, tag="ps")
                y_sb = yp.tile([128, HW], f16, tag="y")
                y_tiles[g] = y_sb
                y2 = tp.tile([128, HW], f16, tag="y2")
                y2_tiles[g] = y2
                tp_c = 0 if g == NCHUNK - 1 else TPOOL
                tp_w[g] = tp_c

                mm_half(psB, B_TILES, HWA, x_sb, osl)
                mm_half(psA, A_TILES, 0, x_sb, osl)

                nc.scalar.activation(
                    out=y_sb[:, HWA:HW],
                    in_=psB[:, 0:HWB],
                    func=mybir.ActivationFunctionType.Copy,
                    accum_out=sums[:, OC + oc : OC + oc + 1],
                )
                nc.scalar.activation(
                    out=y_sb[:, 0:HWA],
                    in_=psA[:, 0:HWA],
                    func=mybir.ActivationFunctionType.Copy,
                    accum_out=sums[:, oc : oc + 1],
                )

                nc.vector.tensor_mul(
                    y2[:, HWA:HW], y_sb[:, HWA:HW], y_sb[:, HWA:HW]
                )
                trash = tp.tile([128, HW], f16, tag="t")
                nc.vector.tensor_scalar(
                    out=trash[:, HWA:HW],
                    in0=y2[:, HWA:HW],
                    scalar1=1.0,
                    scalar2=None,
                    op0=mybir.AluOpType.mult,
                    op1=mybir.AluOpType.add,
                    accum_out=sums[:, 3 * OC + oc : 3 * OC + oc + 1],
                )
                if tp_c:
                    nc.gpsimd.tensor_mul(
                        y2[:, 0:tp_c], y_sb[:, 0:tp_c], y_sb[:, 0:tp_c]
                    )
                return psB

            def square_a_tail(g):
                tp_c = tp_w[g]
                y_sb, y2 = y_tiles[g], y2_tiles[g]
                nc.vector.tensor_mul(
                    y2[:, tp_c:HWA], y_sb[:, tp_c:HWA], y_sb[:, tp_c:HWA]
                )

            def sum_a(g):
                """A-half sum(y^2), one chunk behind its squares."""
                b, oc = divmod(g, OC)
                sums = sums_t[b]
                trash = tp.tile([128, HW], f16, tag="t")
                nc.vector.tensor_scalar(
                    out=trash[:, 0:HWA],
                    in0=y2_tiles.pop(g)[:, 0:HWA],
                    scalar1=1.0,
                    scalar2=None,
                    op0=mybir.AluOpType.mult,
                    op1=mybir.AluOpType.add,
                    accum_out=sums[:, 2 * OC + oc : 2 * OC + oc + 1],
                )

            def emit_chain(b, ps_host):
                """group stats for sample b -> per-channel scale/bias.

                Emitted inside chunk (b+1, 0); the aggregation matmul
                writes into that chunk's own psB tail so the rotating
                PSUM slots never wait on the stats chain."""
                sums = sums_t.pop(b)
                gps = ps_host[:, GPS_OFF : GPS_OFF + 4 * OC]
                nc.tensor.matmul(
                    gps, agg_sb, sums, start=True, stop=True,
                    skip_group_check=True,
                )
                gs = small.tile([128, 4 * OC], f32, tag="gs")
                nc.vector.tensor_copy(out=gs, in_=gps)
                m4 = small.tile([128, OC], f32, tag="m4")
                nc.vector.tensor_add(m4, gs[:, 0:OC], gs[:, OC : 2 * OC])
                q4 = small.tile([128, OC], f32, tag="q4")
                nc.vector.tensor_add(
                    q4, gs[:, 2 * OC : 3 * OC], gs[:, 3 * OC : 4 * OC]
                )
                msq = small.tile([128, OC], f32, tag="msq")
                nc.vector.tensor_mul(msq, m4, m4)
                ve = small.tile([128, OC], f32, tag="ve")
                nc.vector.tensor_sub(ve, q4, msq)
                sd = small.tile([128, OC], f32, tag="sd")
                nc.scalar.activation(
                    out=sd,
                    in_=ve,
                    func=mybir.ActivationFunctionType.Sqrt,
                    bias=eps_sb,
                )
                rstd = small.tile([128, OC], f32, tag="rstd")
                nc.vector.reciprocal(rstd, sd)
                s4 = small.tile([128, OC], f32, tag="s4")
                nc.vector.tensor_mul(s4, rstd, gamma_sb)
                ms = small.tile([128, OC], f32, tag="ms")
                nc.vector.tensor_mul(ms, m4, s4)
                bv4 = small.tile([128, OC], f32, tag="bv4")
                nc.vector.tensor_sub(bv4, beta_sb, ms)
                sb_t[b] = (s4, bv4)

            def emit_transform(g, drain=False):
                """affine + clamp + store. The drain phase splits the
                clamp DVE/Pool (Pool is idle there) with two stores."""
                b, oc = divmod(g, OC)
                s4, bv4 = sb_t[b]
                osl = slice(oc * 128, (oc + 1) * 128)
                u_sb = up.tile([128, HW], f16, tag="u")
                nc.vector.tensor_scalar(
                    out=u_sb,
                    in0=y_tiles.pop(g),
                    scalar1=s4[:, oc : oc + 1],
                    scalar2=bv4[:, oc : oc + 1],
                    op0=mybir.AluOpType.mult,
                    op1=mybir.AluOpType.add,
                )
                f_sb = fp.tile([128, HW], f16, tag="f")
                if drain:
                    nc.gpsimd.tensor_scalar(
                        out=f_sb[:, 0:DRAIN_POOL],
                        in0=u_sb[:, 0:DRAIN_POOL],
                        scalar1=HT_MAX,
                        scalar2=HT_MIN,
                        op0=mybir.AluOpType.min,
                        op1=mybir.AluOpType.max,
                    )
                    nc.vector.tensor_scalar(
                        out=f_sb[:, DRAIN_POOL:HW],
                        in0=u_sb[:, DRAIN_POOL:HW],
                        scalar1=HT_MAX,
                        scalar2=HT_MIN,
                        op0=mybir.AluOpType.min,
                        op1=mybir.AluOpType.max,
                    )
                    nc.sync.dma_start(
                        out=out_d.ap()[b, osl, DRAIN_POOL:HW],
                        in_=f_sb[:, DRAIN_POOL:HW],
                    )
                    nc.sync.dma_start(
                        out=out_d.ap()[b, osl, 0:DRAIN_POOL],
                        in_=f_sb[:, 0:DRAIN_POOL],
                    )
                else:
                    nc.vector.tensor_scalar(
                        out=f_sb,
                        in0=u_sb,
                        scalar1=HT_MAX,
                        scalar2=HT_MIN,
                        op0=mybir.AluOpType.min,
                        op1=mybir.AluOpType.max,
                    )
                    nc.sync.dma_start(out=out_d.ap()[b, osl, :], in_=f_sb)

            # --- main software-pipelined loop over global chunks ----------
            for g in range(NCHUNK):
                b, oc = divmod(g, OC)
                if oc == 0:
                    sums_t[b] = small.tile(
                        [128, 4 * OC], f32, tag="sums", name="sums"
                    )
                if b + 1 < BPC:
                    if oc == 0:
                        xnext = xp.tile(
                            [128, KC, HW], f16, tag="x", name="xnext"
                        )
                        x_tiles.append(xnext)
                        load_x_part(xnext, b + 1, HWA, HW)
                        load_x_part(xnext, b + 1, 0, 1024)
                    elif oc == 1:
                        load_x_part(x_tiles[b + 1], b + 1, 1024, HWA)
                ps_host = emit_chunk(g)
                if g > 0:
                    sum_a(g - 1)
                if oc == 0 and b > 0:
                    # previous sample's stats: agg into THIS chunk's psB
                    emit_chain(b - 1, ps_host)
                if g >= TLAG:
                    emit_transform(g - TLAG)
                square_a_tail(g)
            # --- tail -----------------------------------------------------
            sum_a(NCHUNK - 1)
            emit_chain(BPC - 1, ps_host)
            emit_transform(NCHUNK - TLAG)  # (2, 3)
            for oc in range(OC):
                emit_transform(4 * (BPC - 1) + oc, drain=True)

    nc.compile()
    return nc


def _get_program():
    global _NC_CACHE
    if _NC_CACHE is None:
        _NC_CACHE = _build_program()
    return _NC_CACHE


def _make_in_maps(x, weight, gamma, beta):
    xr = np.ascontiguousarray(x.reshape(B, CIN, HW).astype(np.float16))
    wt = np.ascontiguousarray(weight.T.astype(np.float16))  # [CIN, COUT]
    gamma = np.ascontiguousarray(gamma, dtype=np.float32)
    beta = np.ascontiguousarray(beta, dtype=np.float32)
    agg = np.zeros((128, 128), dtype=np.float32)
    inv = 1.0 / (GSIZE * HW)
    for g in range(128 // GSIZE):
        agg[g * GSIZE : (g + 1) * GSIZE, g * GSIZE : (g + 1) * GSIZE] = inv
    return [
        {
            "x": xr[i * BPC : (i + 1) * BPC],
            "wt": wt,
            "gamma": gamma,
            "beta": beta,
            "agg": agg,
        }
        for i in range(N_CORES)
    ]


def kernel(x, weight, gamma, beta):
    x = np.asarray(x, dtype=np.float32)
    weight = np.asarray(weight, dtype=np.float32)
    assert x.shape == (B, CIN, H, W)
    nc = _get_program()
    in_maps = _make_in_maps(x, weight, gamma, beta)
    res = run_bass_kernel_spmd(nc, in_maps, core_ids=list(range(N_CORES)))
    out = np.concatenate([r["out"] for r in res.results], axis=0)
    return out.astype(np.float32).reshape(B, COUT, H, W)
